# revision 20
# baseline (speedup 1.0000x reference)
"""Trainium2 8-core multi-head attention kernel.

Problem: B=2, S=2048, D=1024, H=16 heads (DK=64), torch-style MHA:
  q/k/v = x @ W*.T + b*;  scores = q k^T / sqrt(DK); mask==0 -> -1e9;
  softmax; ctx = attn @ v; out = ctx @ Wo.T + bo.

Sharding (Megatron-style): core c -> batch b=c//4, head-group hg=c%4
(4 heads = 256 feature dims per core). Each core computes its QKV
projection slices, attention for its heads, and the partial output
projection ctx_hg @ Wo[:, hg].T; the host sums the 4 partials per batch
(the tensor-parallel reduce). bo is folded into the device program via
a ones-row matmul on the hg==0 cores.

Device program notes:
- Activations are fed pre-transposed ([D, S]) so the contraction dim D
  lands on SBUF partitions; weights are fed as W.T slices. This is pure
  layout preparation of the shards.
- All matmuls run in bf16 with f32 PSUM accumulation (f32r moving
  operands measure ~2 cycles/row on HW - bf16 is 2x faster).
- Softmax skips the max-subtraction (scores ~ N(0,1) after the 1/8
  scale folded into the q bias/scale; exp cannot overflow), and the
  mask is applied multiplicatively AFTER exp (exp of a masked score
  times 0 == masked softmax numerator). The denominator comes free from
  the AV matmul via a 65th all-ones column appended to each head's V.
"""

import sys

sys.path.insert(0, "/opt/trn_rl_repo")

from contextlib import ExitStack

import ml_dtypes
import numpy as np

import concourse.bass as bass
import concourse.tile as tile
from concourse import bacc, mybir
from concourse.alu_op_type import AluOpType
from concourse.bass_utils import run_bass_kernel_spmd

P = 128
B, S, D, H = 2, 2048, 1024, 16
DK = 64
NCORES = 8
HG = 4  # head-groups (= cores per batch)
DC = D // HG  # 256 per-core feature dims
HPC = H // HG  # 4 heads per core
KD = D // P  # 8 contraction tiles for projections
SQ = 512  # S_q chunk (matmul moving dim)
NJ = S // SQ  # 4
NI = S // P  # 16 S_k tiles
AFT = mybir.ActivationFunctionType

F32 = mybir.dt.float32
F32R = mybir.dt.float32r
BF16 = mybir.dt.bfloat16

# Projection matmul input dtype (activations + QKV weights). BF16 is 2x
# faster on the PE; F32R (~fp22) is the higher-precision fallback.
PROJ_BF16 = True

_compiled = {}


def _build(dump=False, with_bias=True):
    nc = bacc.Bacc("TRN2", target_bir_lowering=False, debug=False)

    PDT = BF16 if PROJ_BF16 else F32R

    xqT = nc.dram_tensor("xqT", [D, S], PDT, kind="ExternalInput").ap()
    xkT = nc.dram_tensor("xkT", [D, S], PDT, kind="ExternalInput").ap()
    xvT = nc.dram_tensor("xvT", [D, S], PDT, kind="ExternalInput").ap()
    maskT = nc.dram_tensor("maskT", [S, S], BF16, kind="ExternalInput").ap()
    wqT = nc.dram_tensor("wqT", [D, DC], PDT, kind="ExternalInput").ap()
    wkT = nc.dram_tensor("wkT", [D, DC], PDT, kind="ExternalInput").ap()
    wvT = nc.dram_tensor("wvT", [D, DC], PDT, kind="ExternalInput").ap()
    woT = nc.dram_tensor("woT", [DC, D], BF16, kind="ExternalInput").ap()
    bq = nc.dram_tensor("bq", [P, 2], F32, kind="ExternalInput").ap()
    bk = nc.dram_tensor("bk", [P, 2], F32, kind="ExternalInput").ap()
    bv = nc.dram_tensor("bv", [1, DC], BF16, kind="ExternalInput").ap()
    bo = nc.dram_tensor("bo", [1, D], BF16, kind="ExternalInput").ap()
    out = nc.dram_tensor("out", [S, D], F32, kind="ExternalOutput").ap()
    # scratch for softmax-sum reciprocal partition-broadcast (DRAM bounce)
    rscr = nc.dram_tensor("rscr", [16, SQ], F32).ap()

    with tile.TileContext(nc) as tc, ExitStack() as ctx:
        consts = ctx.enter_context(tc.tile_pool(name="consts", bufs=1))
        persist = ctx.enter_context(tc.tile_pool(name="persist", bufs=1))
        xin = ctx.enter_context(tc.tile_pool(name="xin", bufs=12))
        att = ctx.enter_context(tc.tile_pool(name="att", bufs=6))
        mstr = ctx.enter_context(tc.tile_pool(name="mstr", bufs=2))
        nrm = ctx.enter_context(tc.tile_pool(name="nrm", bufs=4))
        outp = ctx.enter_context(tc.tile_pool(name="outp", bufs=3))

        # Constants / weights (DMA'd once)
        wq_sb = consts.tile([P, KD, DC], PDT, name="wq")
        wk_sb = consts.tile([P, KD, DC], PDT, name="wk")
        wv_sb = consts.tile([P, KD, DC], PDT, name="wv")
        nc.sync.dma_start(wq_sb, wqT.rearrange("(k p) c -> p k c", p=P))
        nc.sync.dma_start(wk_sb, wkT.rearrange("(k p) c -> p k c", p=P))
        nc.sync.dma_start(wv_sb, wvT.rearrange("(k p) c -> p k c", p=P))
        wo_sb = consts.tile([P, 2, D], BF16, name="wo")
        nc.sync.dma_start(wo_sb, woT.rearrange("(c p) n -> p c n", p=P))
        bq_sb = consts.tile([P, 2], F32, name="bq")
        bk_sb = consts.tile([P, 2], F32, name="bk")
        nc.sync.dma_start(bq_sb, bq)
        nc.sync.dma_start(bk_sb, bk)
        bv_sb = consts.tile([1, DC], BF16, name="bv")
        bo_sb = consts.tile([1, D], BF16, name="bo")
        nc.sync.dma_start(bv_sb, bv)
        nc.sync.dma_start(bo_sb, bo)
        ones_sb = consts.tile([1, P], BF16, name="ones")
        nc.vector.memset(ones_sb, 1.0)

        # Cross-phase intermediates
        qT_sb = persist.tile([P, 2, S], BF16, name="qT")
        kT_sb = persist.tile([P, 2, S], BF16, name="kT")
        v_sb = persist.tile([P, NI, HPC * (DK + 1)], BF16, name="v")
        ctxT_sb = persist.tile([P, 2, S], BF16, name="ctxT")
        for h in range(HPC):
            nc.vector.memset(v_sb[:, :, 65 * h + 64 : 65 * h + 65], 1.0)

        # ---- Phases A+B overlapped: chunk-0 projections feed attention
        # while chunk-1 projections fill PE gaps during the exp-paced part.
        with (
            tc.tile_pool(name="psA", bufs=2, space="PSUM") as psA,
            tc.tile_pool(name="psSC", bufs=2, space="PSUM") as psSC,
            tc.tile_pool(name="psCX", bufs=2, space="PSUM") as psCX,
        ):
            def proj_qk(c):
                for xdram, w_sb, b_sb, scale, dst in (
                    (xqT, wq_sb, bq_sb, 0.125, qT_sb),
                    (xkT, wk_sb, bk_sb, 1.0, kT_sb),
                ):
                    for j in range(NJ):
                        xt = []
                        for k in range(KD):
                            t = xin.tile([P, SQ], PDT, name="x")
                            nc.gpsimd.dma_start(
                                t, xdram[k * P : (k + 1) * P, j * SQ : (j + 1) * SQ]
                            )
                            xt.append(t)
                        ps = psA.tile([P, SQ], F32, name="ps")
                        for k in range(KD):
                            nc.tensor.matmul(
                                ps,
                                lhsT=w_sb[:, k, c * P : (c + 1) * P],
                                rhs=xt[k],
                                start=(k == 0),
                                stop=(k == KD - 1),
                            )
                        nc.scalar.activation(
                            dst[:, c, j * SQ : (j + 1) * SQ],
                            ps,
                            AFT.Identity,
                            bias=b_sb[:, c : c + 1],
                            scale=scale,
                        )

            def proj_v():
                for j in range(NJ):
                    xt = []
                    for k in range(KD):
                        t = xin.tile([P, SQ], PDT, name="x")
                        nc.gpsimd.dma_start(
                            t, xvT[k * P : (k + 1) * P, j * SQ : (j + 1) * SQ]
                        )
                        xt.append(t)
                    for m in range(SQ // P):
                        i = j * (SQ // P) + m
                        ps = psA.tile([P, SQ], F32, name="ps")
                        for k in range(KD):
                            nc.tensor.matmul(
                                ps[:, :DC],
                                lhsT=xt[k][:, m * P : (m + 1) * P],
                                rhs=wv_sb[:, k, :],
                                start=(k == 0),
                                stop=(not with_bias and k == KD - 1),
                            )
                        if with_bias:
                            nc.tensor.matmul(
                                ps[:, :DC],
                                lhsT=ones_sb,
                                rhs=bv_sb,
                                start=False,
                                stop=True,
                            )
                        for h in range(HPC):
                            nc.scalar.activation(
                                v_sb[:, i, 65 * h : 65 * h + DK],
                                ps[:, DK * h : DK * (h + 1)],
                                AFT.Copy,
                            )

            def attn(c):
                h0, h1 = 2 * c, 2 * c + 1
                for j in range(NJ):
                    mt = mstr.tile([P, NI, SQ], BF16, name="m")
                    nc.gpsimd.dma_start(
                        mt,
                        maskT[:, j * SQ : (j + 1) * SQ].rearrange(
                            "(i p) n -> p i n", p=P
                        ),
                    )
                    ctx0 = psCX.tile([P, SQ], F32, name="ctx")
                    ctx1 = psCX.tile([P, SQ], F32, name="ctx")
                    for i in range(NI):
                        scp = psSC.tile([P, 2, SQ], F32, name="sc")
                        nc.tensor.matmul(
                            scp[:, 0, :],
                            lhsT=kT_sb[0:DK, c, i * P : (i + 1) * P],
                            rhs=qT_sb[0:DK, c, j * SQ : (j + 1) * SQ],
                            start=True,
                            stop=True,
                            tile_position=(0, 0),
                        )
                        nc.tensor.matmul(
                            scp[:, 1, :],
                            lhsT=kT_sb[DK:P, c, i * P : (i + 1) * P],
                            rhs=qT_sb[DK:P, c, j * SQ : (j + 1) * SQ],
                            start=True,
                            stop=True,
                            tile_position=(64, 0),
                        )
                        ap_ = att.tile([P, 2, SQ], BF16, name="a")
                        am = att.tile([P, 2, SQ], BF16, name="am")
                        nc.scalar.activation(ap_, scp, AFT.Exp)
                        msl = mt[:, i, :]
                        nc.vector.tensor_tensor(
                            am,
                            ap_,
                            bass.AP(
                                msl.tensor, msl.offset, [msl.ap[0], [0, 2], msl.ap[1]]
                            ),
                            op=AluOpType.mult,
                        )
                        nc.tensor.matmul(
                            ctx0[: DK + 1],
                            lhsT=v_sb[:, i, 65 * h0 : 65 * h0 + 65],
                            rhs=am[:, 0, :],
                            start=(i == 0),
                            stop=(i == NI - 1),
                        )
                        nc.tensor.matmul(
                            ctx1[: DK + 1],
                            lhsT=v_sb[:, i, 65 * h1 : 65 * h1 + 65],
                            rhs=am[:, 1, :],
                            start=(i == 0),
                            stop=(i == NI - 1),
                        )
                    for hh, cps in ((h0, ctx0), (h1, ctx1)):
                        rbs = nrm.tile([DK, SQ], F32, name="rbs")
                        rb = nrm.tile([DK, SQ], F32, name="rb")
                        scr = rscr[c * 8 + (hh - h0) * 4 + j, :]
                        rc = nrm.tile([P, SQ], F32, name="rc")
                        nc.vector.tensor_copy(rc[DK : DK + 1], cps[DK : DK + 1])
                        nc.sync.dma_start(scr, rc[DK : DK + 1])
                        nc.sync.dma_start(
                            rbs,
                            bass.AP(
                                scr.tensor, scr.offset, [[0, DK]] + scr.ap[-1:]
                            ),
                        )
                        nc.vector.reciprocal_approx_fast(rb, rbs)
                        r0 = DK * (hh % 2)
                        nc.vector.tensor_tensor(
                            ctxT_sb[r0 : r0 + DK, c, j * SQ : (j + 1) * SQ],
                            cps[0:DK],
                            rb,
                            op=AluOpType.mult,
                        )

            proj_qk(0)
            proj_v()
            attn(0)
            proj_qk(1)
            attn(1)

        # ---- Phase C: output projection (partial; host reduces over cores) ----
        with tc.tile_pool(name="psC", bufs=2, space="PSUM") as psC:
            for m in range(NI):
                for n in range(D // SQ):
                    ps = psC.tile([P, SQ], F32, name="po")
                    for hc in range(2):
                        nc.tensor.matmul(
                            ps,
                            lhsT=ctxT_sb[:, hc, m * P : (m + 1) * P],
                            rhs=wo_sb[:, hc, n * SQ : (n + 1) * SQ],
                            start=(hc == 0),
                            stop=(not with_bias and hc == 1),
                        )
                    if with_bias:
                        nc.tensor.matmul(
                            ps,
                            lhsT=ones_sb,
                            rhs=bo_sb[:, n * SQ : (n + 1) * SQ],
                            start=False,
                            stop=True,
                        )
                    ot = outp.tile([P, SQ], F32, name="o")
                    nc.vector.tensor_copy(ot, ps)
                    nc.gpsimd.dma_start(
                        out[m * P : (m + 1) * P, n * SQ : (n + 1) * SQ], ot
                    )

        if dump:
            d_qT = nc.dram_tensor("d_qT", [P, 2, S], F32, kind="ExternalOutput").ap()
            d_kT = nc.dram_tensor("d_kT", [P, 2, S], F32, kind="ExternalOutput").ap()
            d_v = nc.dram_tensor(
                "d_v", [P, NI, HPC * (DK + 1)], BF16, kind="ExternalOutput"
            ).ap()
            d_ctxT = nc.dram_tensor(
                "d_ctxT", [P, 2, S], BF16, kind="ExternalOutput"
            ).ap()
            dpool = ctx.enter_context(tc.tile_pool(name="dump", bufs=1))
            dq = dpool.tile([P, 2, S], F32, name="dq")
            dk = dpool.tile([P, 2, S], F32, name="dk")
            nc.vector.tensor_copy(dq, qT_sb)
            nc.vector.tensor_copy(dk, kT_sb)
            nc.sync.dma_start(d_qT, dq)
            nc.sync.dma_start(d_kT, dk)
            nc.sync.dma_start(d_v, v_sb)
            nc.sync.dma_start(d_ctxT, ctxT_sb)

    nc.compile()
    return nc


def get_nc(with_bias=True):
    if with_bias not in _compiled:
        _compiled[with_bias] = _build(with_bias=with_bias)
    return _compiled[with_bias]


def biases_zero(bq, bk, bv, bo):
    return all(not np.any(np.asarray(x)) for x in (bv, bo))


def make_in_maps(query, key_, value, mask, Wq, bq, Wk, bk, Wv, bv, Wo, bo):
    query = np.asarray(query, dtype=np.float32)
    key_ = np.asarray(key_, dtype=np.float32)
    value = np.asarray(value, dtype=np.float32)
    mask = np.asarray(mask)
    Wq, Wk, Wv, Wo = (np.asarray(w, dtype=np.float32) for w in (Wq, Wk, Wv, Wo))
    bq, bk, bv, bo = (np.asarray(v_, dtype=np.float32) for v_ in (bq, bk, bv, bo))

    pdt = ml_dtypes.bfloat16 if PROJ_BF16 else np.float32

    per_b = []
    for b in range(B):
        per_b.append(
            {
                "xqT": np.ascontiguousarray(query[b].T).astype(pdt),
                "xkT": np.ascontiguousarray(key_[b].T).astype(pdt),
                "xvT": np.ascontiguousarray(value[b].T).astype(pdt),
                "maskT": np.ascontiguousarray(mask[b, 0].T).astype(
                    ml_dtypes.bfloat16
                ),
            }
        )

    in_maps = []
    for c in range(NCORES):
        b, hg = divmod(c, HG)
        sl = slice(DC * hg, DC * (hg + 1))
        m = dict(per_b[b])
        m["wqT"] = np.ascontiguousarray(Wq[sl, :].T).astype(pdt)
        m["wkT"] = np.ascontiguousarray(Wk[sl, :].T).astype(pdt)
        m["wvT"] = np.ascontiguousarray(Wv[sl, :].T).astype(pdt)
        m["woT"] = np.ascontiguousarray(Wo[:, sl].T).astype(ml_dtypes.bfloat16)
        m["bq"] = np.ascontiguousarray((bq[sl] / 8.0).reshape(2, P).T)
        m["bk"] = np.ascontiguousarray(bk[sl].reshape(2, P).T)
        m["bv"] = bv[sl].reshape(1, DC).astype(ml_dtypes.bfloat16)
        m["bo"] = (
            (bo if hg == 0 else np.zeros_like(bo))
            .reshape(1, D)
            .astype(ml_dtypes.bfloat16)
        )
        in_maps.append(m)
    return in_maps


def run(in_maps, with_bias=True, **kwargs):
    nc = get_nc(with_bias=with_bias)
    return run_bass_kernel_spmd(nc, in_maps, core_ids=list(range(NCORES)), **kwargs)


def assemble(results):
    outs = [np.asarray(r["out"], dtype=np.float32) for r in results]
    return np.stack(
        [sum(outs[b * HG : (b + 1) * HG][1:], outs[b * HG]) for b in range(B)]
    )


def kernel(query, key_, value, mask, Wq, bq, Wk, bk, Wv, bv, Wo, bo):
    in_maps = make_in_maps(
        query, key_, value, mask, Wq, bq, Wk, bk, Wv, bv, Wo, bo
    )
    res = run(in_maps, with_bias=not biases_zero(bq, bk, bv, bo))
    return assemble(res.results)


# revision 21
# speedup vs baseline: 1.0283x; 1.0283x over previous
"""Trainium2 8-core multi-head attention kernel.

Problem: B=2, S=2048, D=1024, H=16 heads (DK=64), torch-style MHA:
  q/k/v = x @ W*.T + b*;  scores = q k^T / sqrt(DK); mask==0 -> -1e9;
  softmax; ctx = attn @ v; out = ctx @ Wo.T + bo.

Sharding (Megatron-style): core c -> batch b=c//4, head-group hg=c%4
(4 heads = 256 feature dims per core). Each core computes its QKV
projection slices, attention for its heads, and the partial output
projection ctx_hg @ Wo[:, hg].T; the host sums the 4 partials per batch
(the tensor-parallel reduce). bo is folded into the device program via
a ones-row matmul on the hg==0 cores.

Device program notes:
- Activations are fed pre-transposed ([D, S]) so the contraction dim D
  lands on SBUF partitions; weights are fed as W.T slices. This is pure
  layout preparation of the shards.
- All matmuls run in bf16 with f32 PSUM accumulation (f32r moving
  operands measure ~2 cycles/row on HW - bf16 is 2x faster).
- Softmax skips the max-subtraction (scores ~ N(0,1) after the 1/8
  scale folded into the q bias/scale; exp cannot overflow), and the
  mask is applied multiplicatively AFTER exp (exp of a masked score
  times 0 == masked softmax numerator). The denominator comes free from
  the AV matmul via a 65th all-ones column appended to each head's V.
"""

import sys

sys.path.insert(0, "/opt/trn_rl_repo")

from contextlib import ExitStack

import ml_dtypes
import numpy as np

import concourse.bass as bass
import concourse.tile as tile
from concourse import bacc, mybir
from concourse.alu_op_type import AluOpType
from concourse.bass_utils import run_bass_kernel_spmd

P = 128
B, S, D, H = 2, 2048, 1024, 16
DK = 64
NCORES = 8
HG = 4  # head-groups (= cores per batch)
DC = D // HG  # 256 per-core feature dims
HPC = H // HG  # 4 heads per core
KD = D // P  # 8 contraction tiles for projections
SQ = 512  # S_q chunk (matmul moving dim)
NJ = S // SQ  # 4
NI = S // P  # 16 S_k tiles
AFT = mybir.ActivationFunctionType

F32 = mybir.dt.float32
F32R = mybir.dt.float32r
BF16 = mybir.dt.bfloat16

# Projection matmul input dtype (activations + QKV weights). BF16 is 2x
# faster on the PE; F32R (~fp22) is the higher-precision fallback.
PROJ_BF16 = True

_compiled = {}


def _build(dump=False, with_bias=True):
    nc = bacc.Bacc("TRN2", target_bir_lowering=False, debug=False)

    PDT = BF16 if PROJ_BF16 else F32R

    xqT = nc.dram_tensor("xqT", [D, S], PDT, kind="ExternalInput").ap()
    xkT = nc.dram_tensor("xkT", [D, S], PDT, kind="ExternalInput").ap()
    xvT = nc.dram_tensor("xvT", [D, S], PDT, kind="ExternalInput").ap()
    maskT = nc.dram_tensor("maskT", [S, S], BF16, kind="ExternalInput").ap()
    wqT = nc.dram_tensor("wqT", [D, DC], PDT, kind="ExternalInput").ap()
    wkT = nc.dram_tensor("wkT", [D, DC], PDT, kind="ExternalInput").ap()
    wvT = nc.dram_tensor("wvT", [D, DC], PDT, kind="ExternalInput").ap()
    woT = nc.dram_tensor("woT", [DC, D], BF16, kind="ExternalInput").ap()
    bq = nc.dram_tensor("bq", [P, 2], F32, kind="ExternalInput").ap()
    bk = nc.dram_tensor("bk", [P, 2], F32, kind="ExternalInput").ap()
    bv = nc.dram_tensor("bv", [1, DC], BF16, kind="ExternalInput").ap()
    bo = nc.dram_tensor("bo", [1, D], BF16, kind="ExternalInput").ap()
    out = nc.dram_tensor("out", [S, D], F32, kind="ExternalOutput").ap()
    # scratch for softmax-sum reciprocal partition-broadcast (DRAM bounce)
    rscr = nc.dram_tensor("rscr", [16, SQ], F32).ap()

    with tile.TileContext(nc) as tc, ExitStack() as ctx:
        consts = ctx.enter_context(tc.tile_pool(name="consts", bufs=1))
        persist = ctx.enter_context(tc.tile_pool(name="persist", bufs=1))
        xin = ctx.enter_context(tc.tile_pool(name="xin", bufs=12))
        att = ctx.enter_context(tc.tile_pool(name="att", bufs=6))
        mstr = ctx.enter_context(tc.tile_pool(name="mstr", bufs=2))
        nrm = ctx.enter_context(tc.tile_pool(name="nrm", bufs=4))
        outp = ctx.enter_context(tc.tile_pool(name="outp", bufs=3))

        # Constants / weights (DMA'd once)
        wq_sb = consts.tile([P, KD, DC], PDT, name="wq")
        wk_sb = consts.tile([P, KD, DC], PDT, name="wk")
        wv_sb = consts.tile([P, KD, DC], PDT, name="wv")
        nc.sync.dma_start(wq_sb, wqT.rearrange("(k p) c -> p k c", p=P))
        nc.sync.dma_start(wk_sb, wkT.rearrange("(k p) c -> p k c", p=P))
        nc.sync.dma_start(wv_sb, wvT.rearrange("(k p) c -> p k c", p=P))
        wo_sb = consts.tile([P, 2, D], BF16, name="wo")
        nc.sync.dma_start(wo_sb, woT.rearrange("(c p) n -> p c n", p=P))
        bq_sb = consts.tile([P, 2], F32, name="bq")
        bk_sb = consts.tile([P, 2], F32, name="bk")
        nc.sync.dma_start(bq_sb, bq)
        nc.sync.dma_start(bk_sb, bk)
        bv_sb = consts.tile([1, DC], BF16, name="bv")
        bo_sb = consts.tile([1, D], BF16, name="bo")
        nc.sync.dma_start(bv_sb, bv)
        nc.sync.dma_start(bo_sb, bo)
        ones_sb = consts.tile([1, P], BF16, name="ones")
        nc.vector.memset(ones_sb, 1.0)

        # Cross-phase intermediates
        qT_sb = persist.tile([P, 2, S], BF16, name="qT")
        kT_sb = persist.tile([P, 2, S], BF16, name="kT")
        v_sb = persist.tile([P, NI, HPC * (DK + 1)], BF16, name="v")
        ctxT_sb = persist.tile([P, 2, S], BF16, name="ctxT")
        for h in range(HPC):
            nc.vector.memset(v_sb[:, :, 65 * h + 64 : 65 * h + 65], 1.0)

        # ---- Phases A+B overlapped: chunk-0 projections feed attention
        # while chunk-1 projections fill PE gaps during the exp-paced part.
        with (
            tc.tile_pool(name="psA", bufs=2, space="PSUM") as psA,
            tc.tile_pool(name="psSC", bufs=2, space="PSUM") as psSC,
            tc.tile_pool(name="psCX", bufs=2, space="PSUM") as psCX,
        ):
            def proj_qk(c):
                for xdram, w_sb, b_sb, scale, dst in (
                    (xqT, wq_sb, bq_sb, 0.125, qT_sb),
                    (xkT, wk_sb, bk_sb, 1.0, kT_sb),
                ):
                    for j in range(NJ):
                        xt = []
                        for k in range(KD):
                            t = xin.tile([P, SQ], PDT, name="x")
                            nc.gpsimd.dma_start(
                                t, xdram[k * P : (k + 1) * P, j * SQ : (j + 1) * SQ]
                            )
                            xt.append(t)
                        ps = psA.tile([P, SQ], F32, name="ps")
                        for k in range(KD):
                            nc.tensor.matmul(
                                ps,
                                lhsT=w_sb[:, k, c * P : (c + 1) * P],
                                rhs=xt[k],
                                start=(k == 0),
                                stop=(k == KD - 1),
                            )
                        nc.scalar.activation(
                            dst[:, c, j * SQ : (j + 1) * SQ],
                            ps,
                            AFT.Identity,
                            bias=b_sb[:, c : c + 1],
                            scale=scale,
                        )

            def proj_v():
                for j in range(NJ):
                    xt = []
                    for k in range(KD):
                        t = xin.tile([P, SQ], PDT, name="x")
                        nc.gpsimd.dma_start(
                            t, xvT[k * P : (k + 1) * P, j * SQ : (j + 1) * SQ]
                        )
                        xt.append(t)
                    for m in range(SQ // P):
                        i = j * (SQ // P) + m
                        ps = psA.tile([P, SQ], F32, name="ps")
                        for k in range(KD):
                            nc.tensor.matmul(
                                ps[:, :DC],
                                lhsT=xt[k][:, m * P : (m + 1) * P],
                                rhs=wv_sb[:, k, :],
                                start=(k == 0),
                                stop=(not with_bias and k == KD - 1),
                            )
                        if with_bias:
                            nc.tensor.matmul(
                                ps[:, :DC],
                                lhsT=ones_sb,
                                rhs=bv_sb,
                                start=False,
                                stop=True,
                            )
                        for h in range(HPC):
                            nc.scalar.activation(
                                v_sb[:, i, 65 * h : 65 * h + DK],
                                ps[:, DK * h : DK * (h + 1)],
                                AFT.Copy,
                            )

            def attn(c):
                h0, h1 = 2 * c, 2 * c + 1
                for j in range(NJ):
                    mt = mstr.tile([P, NI, SQ], BF16, name="m")
                    nc.gpsimd.dma_start(
                        mt,
                        maskT[:, j * SQ : (j + 1) * SQ].rearrange(
                            "(i p) n -> p i n", p=P
                        ),
                    )
                    ctx0 = psCX.tile([P, SQ], F32, name="ctx")
                    ctx1 = psCX.tile([P, SQ], F32, name="ctx")
                    for i in range(NI):
                        scp = psSC.tile([P, 2, SQ], F32, name="sc")
                        nc.tensor.matmul(
                            scp[:, 0, :],
                            lhsT=kT_sb[0:DK, c, i * P : (i + 1) * P],
                            rhs=qT_sb[0:DK, c, j * SQ : (j + 1) * SQ],
                            start=True,
                            stop=True,
                            tile_position=(0, 0),
                        )
                        nc.tensor.matmul(
                            scp[:, 1, :],
                            lhsT=kT_sb[DK:P, c, i * P : (i + 1) * P],
                            rhs=qT_sb[DK:P, c, j * SQ : (j + 1) * SQ],
                            start=True,
                            stop=True,
                            tile_position=(64, 0),
                        )
                        ap_ = att.tile([P, 2, SQ], BF16, name="a")
                        am = att.tile([P, 2, SQ], BF16, name="am")
                        nc.scalar.activation(ap_, scp, AFT.Exp)
                        msl = mt[:, i, :]
                        nc.vector.tensor_tensor(
                            am,
                            ap_,
                            bass.AP(
                                msl.tensor, msl.offset, [msl.ap[0], [0, 2], msl.ap[1]]
                            ),
                            op=AluOpType.mult,
                        )
                        nc.tensor.matmul(
                            ctx0[: DK + 1],
                            lhsT=v_sb[:, i, 65 * h0 : 65 * h0 + 65],
                            rhs=am[:, 0, :],
                            start=(i == 0),
                            stop=(i == NI - 1),
                        )
                        nc.tensor.matmul(
                            ctx1[: DK + 1],
                            lhsT=v_sb[:, i, 65 * h1 : 65 * h1 + 65],
                            rhs=am[:, 1, :],
                            start=(i == 0),
                            stop=(i == NI - 1),
                        )
                    for hh, cps in ((h0, ctx0), (h1, ctx1)):
                        # one DVE copy frees the PSUM ctx bank immediately;
                        # the normalize chain then runs off the critical path
                        cb = nrm.tile([P, SQ], F32, name="cb")
                        nc.vector.tensor_copy(cb[0 : DK + 1], cps[0 : DK + 1])
                        rbs = nrm.tile([DK, SQ], F32, name="rbs")
                        rb = nrm.tile([DK, SQ], F32, name="rb")
                        scr = rscr[c * 8 + (hh - h0) * 4 + j, :]
                        nc.sync.dma_start(scr, cb[DK : DK + 1])
                        nc.sync.dma_start(
                            rbs,
                            bass.AP(
                                scr.tensor, scr.offset, [[0, DK]] + scr.ap[-1:]
                            ),
                        )
                        nc.vector.reciprocal_approx_fast(rb, rbs)
                        r0 = DK * (hh % 2)
                        nc.vector.tensor_tensor(
                            ctxT_sb[r0 : r0 + DK, c, j * SQ : (j + 1) * SQ],
                            cb[0:DK],
                            rb,
                            op=AluOpType.mult,
                        )

            proj_qk(0)
            proj_v()
            attn(0)
            proj_qk(1)
            attn(1)

        # ---- Phase C: output projection (partial; host reduces over cores) ----
        with tc.tile_pool(name="psC", bufs=2, space="PSUM") as psC:
            for m in range(NI):
                for n in range(D // SQ):
                    ps = psC.tile([P, SQ], F32, name="po")
                    for hc in range(2):
                        nc.tensor.matmul(
                            ps,
                            lhsT=ctxT_sb[:, hc, m * P : (m + 1) * P],
                            rhs=wo_sb[:, hc, n * SQ : (n + 1) * SQ],
                            start=(hc == 0),
                            stop=(not with_bias and hc == 1),
                        )
                    if with_bias:
                        nc.tensor.matmul(
                            ps,
                            lhsT=ones_sb,
                            rhs=bo_sb[:, n * SQ : (n + 1) * SQ],
                            start=False,
                            stop=True,
                        )
                    ot = outp.tile([P, SQ], F32, name="o")
                    nc.vector.tensor_copy(ot, ps)
                    nc.gpsimd.dma_start(
                        out[m * P : (m + 1) * P, n * SQ : (n + 1) * SQ], ot
                    )

        if dump:
            d_qT = nc.dram_tensor("d_qT", [P, 2, S], F32, kind="ExternalOutput").ap()
            d_kT = nc.dram_tensor("d_kT", [P, 2, S], F32, kind="ExternalOutput").ap()
            d_v = nc.dram_tensor(
                "d_v", [P, NI, HPC * (DK + 1)], BF16, kind="ExternalOutput"
            ).ap()
            d_ctxT = nc.dram_tensor(
                "d_ctxT", [P, 2, S], BF16, kind="ExternalOutput"
            ).ap()
            dpool = ctx.enter_context(tc.tile_pool(name="dump", bufs=1))
            dq = dpool.tile([P, 2, S], F32, name="dq")
            dk = dpool.tile([P, 2, S], F32, name="dk")
            nc.vector.tensor_copy(dq, qT_sb)
            nc.vector.tensor_copy(dk, kT_sb)
            nc.sync.dma_start(d_qT, dq)
            nc.sync.dma_start(d_kT, dk)
            nc.sync.dma_start(d_v, v_sb)
            nc.sync.dma_start(d_ctxT, ctxT_sb)

    nc.compile()
    return nc


def get_nc(with_bias=True):
    if with_bias not in _compiled:
        _compiled[with_bias] = _build(with_bias=with_bias)
    return _compiled[with_bias]


def biases_zero(bq, bk, bv, bo):
    return all(not np.any(np.asarray(x)) for x in (bv, bo))


def make_in_maps(query, key_, value, mask, Wq, bq, Wk, bk, Wv, bv, Wo, bo):
    query = np.asarray(query, dtype=np.float32)
    key_ = np.asarray(key_, dtype=np.float32)
    value = np.asarray(value, dtype=np.float32)
    mask = np.asarray(mask)
    Wq, Wk, Wv, Wo = (np.asarray(w, dtype=np.float32) for w in (Wq, Wk, Wv, Wo))
    bq, bk, bv, bo = (np.asarray(v_, dtype=np.float32) for v_ in (bq, bk, bv, bo))

    pdt = ml_dtypes.bfloat16 if PROJ_BF16 else np.float32

    per_b = []
    for b in range(B):
        per_b.append(
            {
                "xqT": np.ascontiguousarray(query[b].T).astype(pdt),
                "xkT": np.ascontiguousarray(key_[b].T).astype(pdt),
                "xvT": np.ascontiguousarray(value[b].T).astype(pdt),
                "maskT": np.ascontiguousarray(mask[b, 0].T).astype(
                    ml_dtypes.bfloat16
                ),
            }
        )

    in_maps = []
    for c in range(NCORES):
        b, hg = divmod(c, HG)
        sl = slice(DC * hg, DC * (hg + 1))
        m = dict(per_b[b])
        m["wqT"] = np.ascontiguousarray(Wq[sl, :].T).astype(pdt)
        m["wkT"] = np.ascontiguousarray(Wk[sl, :].T).astype(pdt)
        m["wvT"] = np.ascontiguousarray(Wv[sl, :].T).astype(pdt)
        m["woT"] = np.ascontiguousarray(Wo[:, sl].T).astype(ml_dtypes.bfloat16)
        m["bq"] = np.ascontiguousarray((bq[sl] / 8.0).reshape(2, P).T)
        m["bk"] = np.ascontiguousarray(bk[sl].reshape(2, P).T)
        m["bv"] = bv[sl].reshape(1, DC).astype(ml_dtypes.bfloat16)
        m["bo"] = (
            (bo if hg == 0 else np.zeros_like(bo))
            .reshape(1, D)
            .astype(ml_dtypes.bfloat16)
        )
        in_maps.append(m)
    return in_maps


def run(in_maps, with_bias=True, **kwargs):
    nc = get_nc(with_bias=with_bias)
    return run_bass_kernel_spmd(nc, in_maps, core_ids=list(range(NCORES)), **kwargs)


def assemble(results):
    outs = [np.asarray(r["out"], dtype=np.float32) for r in results]
    return np.stack(
        [sum(outs[b * HG : (b + 1) * HG][1:], outs[b * HG]) for b in range(B)]
    )


def kernel(query, key_, value, mask, Wq, bq, Wk, bk, Wv, bv, Wo, bo):
    in_maps = make_in_maps(
        query, key_, value, mask, Wq, bq, Wk, bk, Wv, bv, Wo, bo
    )
    res = run(in_maps, with_bias=not biases_zero(bq, bk, bv, bo))
    return assemble(res.results)
